# revision 13
# baseline (speedup 1.0000x reference)
"""Trainium2 Bass kernel for nn_IntervalClusterTripletFT (retrieval_knn).

Strategy (sharding_hint): shard the anchor (row) dimension of the NxN
distance matrix across 8 cores; all embeddings replicated on every core;
each core mines its own rows and computes local triplet-loss terms; host
combines the 8 partial sums into the mean.

Math: for anchors i and candidates j with pos-window W(i) (i's cluster):
    d2(i,j) = sq_i + sq_j - 2*G_ij = sq_i - 2*v(i,j),  v = G - sq_j/2
    hardest-pos  hp_i = sqrt(sq_i - 2*min_{j in W} v)
    hardest-neg  hn_i = sqrt(sq_i - 2*max_{j not in W} v)
    loss = mean(relu(hp - hn + 1))
v is produced directly in PSUM by a K=257 accumulated matmul: two K=128
fp32r passes over embT plus one K=1 pass (ones x (-sq/2) row).  Each core
gets a row-rotated copy of the data (np.roll by -512*core) so its own 512
anchors sit in columns 0..511: the program is identical on all cores.
The eps (1e-6) the reference adds inside the triplet norm is dropped;
measured effect on the loss is ~1.2e-6 relative.
"""

import sys

sys.path.insert(0, "/opt/trn_rl_repo")

import numpy as np

C, K, D = 256, 16, 256
N = C * K              # 4096 embeddings
NCORES = 8
ROWS = N // NCORES     # 512 anchor rows per core
RT = ROWS // 128       # 4 row-tiles of 128 anchors
NCH = N // 512         # 8 column chunks of 512 candidates
HALF = 2048            # columns per PSUM half (4 banks)
BIG = 1.0e30

TRACE = False          # test.py sets this for profiled runs
import os as _os

STAGE = int(_os.environ.get("KSTAGE", "3"))  # bisection: 1=mm+reduce 2=+ttr 3=full
KSUB = _os.environ.get("KSUB", "both")
_CACHE = {}


def _build_nc():
    from contextlib import ExitStack

    import concourse.bacc as bacc
    import concourse.mybir as mybir
    import concourse.tile as tile

    fr = mybir.dt.float32r
    f32 = mybir.dt.float32
    Alu = mybir.AluOpType
    Act = mybir.ActivationFunctionType
    AxX = mybir.AxisListType.X

    nc = bacc.Bacc(
        "TRN2",
        target_bir_lowering=False,
        debug=False,
        num_devices=NCORES,
    )
    xt0 = nc.dram_tensor("xt0", [128, N], f32, kind="ExternalInput").ap()
    xt1 = nc.dram_tensor("xt1", [128, N], f32, kind="ExternalInput").ap()
    sqr = nc.dram_tensor("sqr", [1, N], f32, kind="ExternalInput").ap()
    sqp = nc.dram_tensor("sqp", [128, RT], f32, kind="ExternalInput").ap()
    mpos = nc.dram_tensor("mpos", [128, 128], f32, kind="ExternalInput").ap()
    mneg = nc.dram_tensor("mneg", [128, 128], f32, kind="ExternalInput").ap()
    onesd = nc.dram_tensor("onesd", [1, 128], f32, kind="ExternalInput").ap()
    outd = nc.dram_tensor("lossv", [128, RT], f32, kind="ExternalOutput").ap()

    with tile.TileContext(nc) as tc, ExitStack() as ctx:
        const = ctx.enter_context(tc.tile_pool(name="const", bufs=1))
        psum = ctx.enter_context(tc.tile_pool(name="psum", bufs=2, space="PSUM"))
        work = ctx.enter_context(tc.tile_pool(name="work", bufs=2))

        e0 = [
            const.tile([128, 512], fr, tag=f"e0_{j}", name=f"e0_{j}")
            for j in range(NCH)
        ]
        e1 = [
            const.tile([128, 512], fr, tag=f"e1_{j}", name=f"e1_{j}")
            for j in range(NCH)
        ]
        sqt = const.tile([1, N], fr, tag="sqt")
        sqpt = const.tile([128, RT], f32, tag="sqpt")
        mpt = const.tile([128, 128], f32, tag="mpt")
        mnt = const.tile([128, 128], f32, tag="mnt")
        ones = const.tile([1, 128], fr, tag="ones")
        minw = const.tile([128, RT], f32, tag="minw")
        maxa = const.tile([128, RT], f32, tag="maxa")
        maxb = const.tile([128, RT], f32, tag="maxb")
        hpn2 = const.tile([128, 2 * RT], f32, tag="hpn2")
        hpn = const.tile([128, 2 * RT], f32, tag="hpn")
        diff = const.tile([128, RT], f32, tag="diff")
        lossv = const.tile([128, RT], f32, tag="lossv_sb")

        nc.gpsimd.dma_start(ones[:], onesd[:])
        nc.gpsimd.dma_start(sqt[:], sqr[:])
        nc.sync.dma_start(sqpt[:], sqp[:])
        nc.sync.dma_start(mpt[:], mpos[:])
        nc.sync.dma_start(mnt[:], mneg[:])
        for j in range(NCH):
            sl = slice(512 * j, 512 * (j + 1))
            nc.gpsimd.dma_start(e0[j][:], xt0[:, sl])
            nc.gpsimd.dma_start(e1[j][:], xt1[:, sl])

        for lt in range(RT):
            rsl = slice(128 * lt, 128 * lt + 128)  # own-anchor cols (in chunk 0)
            for h in range(2):
                pt = psum.tile([128, HALF], f32, tag="acc")
                # k-outer: reuse each stationary operand across the half's
                # 4 chunks before switching (fewer PE weight reloads)
                for ki in range(3):
                    for jj in range(4):
                        j = 4 * h + jj
                        csl = slice(512 * j, 512 * (j + 1))
                        osl = slice(512 * jj, 512 * (jj + 1))
                        if ki == 0:
                            nc.tensor.matmul(
                                pt[:, osl], e0[0][:, rsl], e0[j][:],
                                start=True, stop=False,
                            )
                        elif ki == 1:
                            nc.tensor.matmul(
                                pt[:, osl], e1[0][:, rsl], e1[j][:],
                                start=False, stop=False,
                            )
                        else:
                            nc.tensor.matmul(
                                pt[:, osl], ones[:], sqt[:, csl],
                                start=False, stop=True,
                            )
                if h == 0 and STAGE >= 2:
                    dsl = slice(128 * lt, 128 * lt + 128)
                    scr = work.tile([128, 128], f32, tag="scr")
                    # hardest-pos: min of v over the window (off-window -> +BIG)
                    nc.vector.tensor_tensor(scr[:], pt[:, dsl], mpt[:], Alu.add)
                    nc.vector.tensor_reduce(
                        minw[:, lt : lt + 1], scr[:], axis=AxX, op=Alu.min
                    )
                    # suppress the window for the neg-max
                    nc.vector.tensor_tensor(pt[:, dsl], pt[:, dsl], mnt[:], Alu.add)
                if h == 0:
                    nc.vector.tensor_reduce(
                        maxa[:, lt : lt + 1], pt[:], axis=AxX, op=Alu.max
                    )
                else:
                    nc.vector.tensor_reduce(
                        maxb[:, lt : lt + 1], pt[:], axis=AxX, op=Alu.max
                    )

        # tail: hp/hn and per-anchor loss terms
        if STAGE >= 3:
            mx = work.tile([128, RT], f32, tag="mx")
            nc.vector.tensor_max(mx[:], maxa[:], maxb[:])
            nc.vector.scalar_tensor_tensor(
                hpn2[:, 0:RT], minw[:], -2.0, sqpt[:], Alu.mult, Alu.add
            )
            nc.vector.scalar_tensor_tensor(
                hpn2[:, RT : 2 * RT], mx[:], -2.0, sqpt[:], Alu.mult, Alu.add
            )
            nc.scalar.activation(hpn[:], hpn2[:], Act.Sqrt)
            nc.vector.tensor_sub(diff[:], hpn[:, 0:RT], hpn[:, RT : 2 * RT])
            nc.vector.tensor_scalar(
                lossv[:], diff[:], 1.0, 0.0, op0=Alu.add, op1=Alu.max
            )
            nc.sync.dma_start(outd[:], lossv[:])
        else:
            nc.vector.tensor_max(lossv[:], maxa[:], maxb[:])
            nc.sync.dma_start(outd[:], lossv[:])

    nc.compile()  # bacc register allocation / DCE — required before walrus
    return nc


def _prep_inputs(batch):
    emb = np.ascontiguousarray(batch.reshape(N, D).astype(np.float32))
    sq = np.einsum("nd,nd->n", emb, emb).astype(np.float32)
    blk = np.kron(np.eye(8, dtype=bool), np.ones((16, 16), dtype=bool))
    mpos = np.where(blk, np.float32(0.0), np.float32(BIG)).astype(np.float32)
    mneg = np.where(blk, np.float32(-BIG), np.float32(0.0)).astype(np.float32)
    in_maps = []
    for c in range(NCORES):
        rot = np.roll(emb, -ROWS * c, axis=0)
        sqrot = np.roll(sq, -ROWS * c)
        xt = np.ascontiguousarray(rot.T)  # [D, N]
        in_maps.append(
            {
                "xt0": np.ascontiguousarray(xt[0:128]),
                "xt1": np.ascontiguousarray(xt[128:256]),
                "sqr": np.ascontiguousarray((-0.5 * sqrot)[None, :].astype(np.float32)),
                "sqp": np.ascontiguousarray(
                    sqrot[:ROWS].reshape(RT, 128).T.astype(np.float32)
                ),
                "mpos": mpos,
                "mneg": mneg,
                "onesd": np.ones((1, 128), dtype=np.float32),
            }
        )
    return in_maps


def kernel(batch):
    batch = np.asarray(batch)
    in_maps = _prep_inputs(batch)
    if "nc" not in _CACHE:
        _CACHE["nc"] = _build_nc()
    nc = _CACHE["nc"]

    from concourse.bass_utils import run_bass_kernel_spmd

    res = run_bass_kernel_spmd(
        nc, in_maps, core_ids=list(range(NCORES)), trace=TRACE
    )
    _CACHE["last_result"] = res
    total = np.float64(0.0)
    for r in res.results:
        total += np.float64(r["lossv"].astype(np.float64).sum())
    return np.array(total / N, dtype=np.float32)


# revision 14
# speedup vs baseline: 1.1011x; 1.1011x over previous
"""Trainium2 Bass kernel for nn_IntervalClusterTripletFT (retrieval_knn).

Strategy (sharding_hint): shard the anchor (row) dimension of the NxN
distance matrix across 8 cores; all embeddings replicated on every core;
each core mines its own rows and computes local triplet-loss terms; host
combines the 8 partial sums into the mean.

Math: for anchors i and candidates j with pos-window W(i) (i's cluster):
    d2(i,j) = sq_i + sq_j - 2*G_ij = sq_i - 2*v(i,j),  v = G - sq_j/2
    hardest-pos  hp_i = sqrt(sq_i - 2*min_{j in W} v)
    hardest-neg  hn_i = sqrt(sq_i - 2*max_{j not in W} v)
    loss = mean(relu(hp - hn + 1))
v is produced directly in PSUM by a K=257 accumulated matmul: two K=128
fp32r passes over embT plus one K=1 pass (ones x (-sq/2) row).  Each core
gets a row-rotated copy of the data (np.roll by -512*core) so its own 512
anchors sit in columns 0..511: the program is identical on all cores.
The eps (1e-6) the reference adds inside the triplet norm is dropped;
measured effect on the loss is ~1.2e-6 relative.
"""

import sys

sys.path.insert(0, "/opt/trn_rl_repo")

import numpy as np

C, K, D = 256, 16, 256
N = C * K              # 4096 embeddings
NCORES = 8
ROWS = N // NCORES     # 512 anchor rows per core
RT = ROWS // 128       # 4 row-tiles of 128 anchors
NCH = N // 512         # 8 column chunks of 512 candidates
HALF = 1024            # columns per PSUM quarter (2 banks)
BIG = 1.0e30

TRACE = False          # test.py sets this for profiled runs
import os as _os

STAGE = int(_os.environ.get("KSTAGE", "3"))  # bisection: 1=mm+reduce 2=+ttr 3=full
KSUB = _os.environ.get("KSUB", "both")
_CACHE = {}


def _build_nc():
    from contextlib import ExitStack

    import concourse.bacc as bacc
    import concourse.mybir as mybir
    import concourse.tile as tile

    fr = mybir.dt.float32r
    f32 = mybir.dt.float32
    Alu = mybir.AluOpType
    Act = mybir.ActivationFunctionType
    AxX = mybir.AxisListType.X

    nc = bacc.Bacc(
        "TRN2",
        target_bir_lowering=False,
        debug=False,
        num_devices=NCORES,
    )
    xt0 = nc.dram_tensor("xt0", [128, N], f32, kind="ExternalInput").ap()
    xt1 = nc.dram_tensor("xt1", [128, N], f32, kind="ExternalInput").ap()
    sqr = nc.dram_tensor("sqr", [1, N], f32, kind="ExternalInput").ap()
    sqp = nc.dram_tensor("sqp", [128, RT], f32, kind="ExternalInput").ap()
    mpos = nc.dram_tensor("mpos", [128, 128], f32, kind="ExternalInput").ap()
    mneg = nc.dram_tensor("mneg", [128, 128], f32, kind="ExternalInput").ap()
    onesd = nc.dram_tensor("onesd", [1, 128], f32, kind="ExternalInput").ap()
    outd = nc.dram_tensor("lossv", [128, RT], f32, kind="ExternalOutput").ap()

    with tile.TileContext(nc) as tc, ExitStack() as ctx:
        const = ctx.enter_context(tc.tile_pool(name="const", bufs=1))
        psum = ctx.enter_context(tc.tile_pool(name="psum", bufs=4, space="PSUM"))
        work = ctx.enter_context(tc.tile_pool(name="work", bufs=2))

        e0 = [
            const.tile([128, 512], fr, tag=f"e0_{j}", name=f"e0_{j}")
            for j in range(NCH)
        ]
        e1 = [
            const.tile([128, 512], fr, tag=f"e1_{j}", name=f"e1_{j}")
            for j in range(NCH)
        ]
        sqt = const.tile([1, N], fr, tag="sqt")
        sqpt = const.tile([128, RT], f32, tag="sqpt")
        mpt = const.tile([128, 128], f32, tag="mpt")
        mnt = const.tile([128, 128], f32, tag="mnt")
        ones = const.tile([1, 128], fr, tag="ones")
        minw = const.tile([128, RT], f32, tag="minw")
        maxq = const.tile([128, 4 * RT], f32, tag="maxq")
        hpn2 = const.tile([128, 2 * RT], f32, tag="hpn2")
        hpn = const.tile([128, 2 * RT], f32, tag="hpn")
        diff = const.tile([128, RT], f32, tag="diff")
        lossv = const.tile([128, RT], f32, tag="lossv_sb")

        nc.gpsimd.dma_start(ones[:], onesd[:])
        nc.gpsimd.dma_start(sqt[:], sqr[:])
        nc.sync.dma_start(sqpt[:], sqp[:])
        nc.sync.dma_start(mpt[:], mpos[:])
        nc.sync.dma_start(mnt[:], mneg[:])
        for j in range(NCH):
            sl = slice(512 * j, 512 * (j + 1))
            nc.gpsimd.dma_start(e0[j][:], xt0[:, sl])
            nc.gpsimd.dma_start(e1[j][:], xt1[:, sl])

        for lt in range(RT):
            rsl = slice(128 * lt, 128 * lt + 128)  # own-anchor cols (in chunk 0)
            for h in range(4):
                pt = psum.tile([128, HALF], f32, tag="acc")
                # k-outer: reuse each stationary operand across the quarter's
                # chunks before switching (fewer PE weight reloads)
                for ki in range(3):
                    for jj in range(2):
                        j = 2 * h + jj
                        csl = slice(512 * j, 512 * (j + 1))
                        osl = slice(512 * jj, 512 * (jj + 1))
                        if ki == 0:
                            nc.tensor.matmul(
                                pt[:, osl], e0[0][:, rsl], e0[j][:],
                                start=True, stop=False,
                            )
                        elif ki == 1:
                            nc.tensor.matmul(
                                pt[:, osl], e1[0][:, rsl], e1[j][:],
                                start=False, stop=False,
                            )
                        else:
                            nc.tensor.matmul(
                                pt[:, osl], ones[:], sqt[:, csl],
                                start=False, stop=True,
                            )
                if h == 0 and STAGE >= 2:
                    dsl = slice(128 * lt, 128 * lt + 128)
                    scr = work.tile([128, 128], f32, tag="scr")
                    # hardest-pos: min of v over the window (off-window -> +BIG)
                    nc.vector.tensor_tensor(scr[:], pt[:, dsl], mpt[:], Alu.add)
                    nc.vector.tensor_reduce(
                        minw[:, lt : lt + 1], scr[:], axis=AxX, op=Alu.min
                    )
                    # suppress the window for the neg-max
                    nc.vector.tensor_tensor(pt[:, dsl], pt[:, dsl], mnt[:], Alu.add)
                qc = 4 * lt + h
                nc.vector.tensor_reduce(
                    maxq[:, qc : qc + 1], pt[:], axis=AxX, op=Alu.max
                )

        # tail: hp/hn and per-anchor loss terms
        if STAGE >= 3:
            mx = work.tile([128, RT], f32, tag="mx")
            nc.vector.tensor_reduce(
                mx[:], maxq[:].rearrange("p (t q) -> p t q", q=4), axis=AxX, op=Alu.max
            )
            nc.vector.scalar_tensor_tensor(
                hpn2[:, 0:RT], minw[:], -2.0, sqpt[:], Alu.mult, Alu.add
            )
            nc.vector.scalar_tensor_tensor(
                hpn2[:, RT : 2 * RT], mx[:], -2.0, sqpt[:], Alu.mult, Alu.add
            )
            nc.scalar.activation(hpn[:], hpn2[:], Act.Sqrt)
            nc.vector.tensor_sub(diff[:], hpn[:, 0:RT], hpn[:, RT : 2 * RT])
            nc.vector.tensor_scalar(
                lossv[:], diff[:], 1.0, 0.0, op0=Alu.add, op1=Alu.max
            )
            nc.sync.dma_start(outd[:], lossv[:])
        else:
            nc.vector.tensor_copy(lossv[:], maxq[:, 0:RT])
            nc.sync.dma_start(outd[:], lossv[:])

    nc.compile()  # bacc register allocation / DCE — required before walrus
    return nc


def _prep_inputs(batch):
    emb = np.ascontiguousarray(batch.reshape(N, D).astype(np.float32))
    sq = np.einsum("nd,nd->n", emb, emb).astype(np.float32)
    blk = np.kron(np.eye(8, dtype=bool), np.ones((16, 16), dtype=bool))
    mpos = np.where(blk, np.float32(0.0), np.float32(BIG)).astype(np.float32)
    mneg = np.where(blk, np.float32(-BIG), np.float32(0.0)).astype(np.float32)
    in_maps = []
    for c in range(NCORES):
        rot = np.roll(emb, -ROWS * c, axis=0)
        sqrot = np.roll(sq, -ROWS * c)
        xt = np.ascontiguousarray(rot.T)  # [D, N]
        in_maps.append(
            {
                "xt0": np.ascontiguousarray(xt[0:128]),
                "xt1": np.ascontiguousarray(xt[128:256]),
                "sqr": np.ascontiguousarray((-0.5 * sqrot)[None, :].astype(np.float32)),
                "sqp": np.ascontiguousarray(
                    sqrot[:ROWS].reshape(RT, 128).T.astype(np.float32)
                ),
                "mpos": mpos,
                "mneg": mneg,
                "onesd": np.ones((1, 128), dtype=np.float32),
            }
        )
    return in_maps


def kernel(batch):
    batch = np.asarray(batch)
    in_maps = _prep_inputs(batch)
    if "nc" not in _CACHE:
        _CACHE["nc"] = _build_nc()
    nc = _CACHE["nc"]

    from concourse.bass_utils import run_bass_kernel_spmd

    res = run_bass_kernel_spmd(
        nc, in_maps, core_ids=list(range(NCORES)), trace=TRACE
    )
    _CACHE["last_result"] = res
    total = np.float64(0.0)
    for r in res.results:
        total += np.float64(r["lossv"].astype(np.float64).sum())
    return np.array(total / N, dtype=np.float32)
